# revision 59
# baseline (speedup 1.0000x reference)
"""AllAtomFAPE loss kernel for Trainium2 (8 NeuronCores, SPMD).

Problem: b=1, N=384 res, F=8 frames/res -> NF=3072 frames; A=14 atoms/res
-> NA=5376 atoms. Output: scalar (shape (1,)) masked clamped FAPE.

Algorithm (K=28 Gram factorization, host-precomputed features):
  lp - lt = A_f x_a with A_f = [pR^T | -tR^T | -w_f] (3x7),
  w_f = pR^T pt - tR^T tt, x_a = [pp; tp; 1] (7,). So
    d2(f,a) = x_a^T G_f x_a,  G_f = A_f^T A_f (7x7 PSD)
  which is a K=28 dot product between frame features W[:,f] (the 28
  unique entries of G, off-diagonals doubled) and atom features Z[:,a]
  (the matching monomials of x_a). Both feature slabs are computed on
  the host in float64 and rounded to bf16; the device does only the
  O(NF*NA) pairwise work, pipelined per (block, group) tile:
    PE   : d2 tile = W_blk^T @ Z  (bf16 matmul, f32 PSUM, 512-col moving)
    DVE  : clamp(d2, 0, 100) -> bf16 block buffer (clamp commutes with
           sqrt: min(sqrt(x+eps),10) = sqrt(min(x,100)+eps); the
           max(.,0) guards bf16-rounding-induced negative d2 from
           NaN-ing the sqrt). This pass is the ~19us critical stream;
           it must run on DVE (GpSimd cannot read PSUM, ACT cannot
           clamp) at 1 elem/cycle (f32 input forbids the 2x modes).
    ACT  : sqrt(x + eps) + per-partition accumulate, over half-block
           spans (per-tile spans in the last block shorten the tail)
  Epilogue: DMA the raw [128, 8] accumulator columns per core straight
  from SBUF (no on-device reduce -- shortens the serial tail); the
  host's unshard step sums the 8 per-core slabs and normalizes (binary
  atom masks fold in by zeroing Z columns + a sqrt(eps) correction).
  Steady state is PE-throughput-bound (~600ns per 512-col bf16 matmul
  + ldweights reload per call); fp8 DoubleRow would halve that but
  costs a systematic fp8-rounding bias in d^2.

Sharding: frames sharded across the 8 cores (384 each = 3 stationary
blocks of 128); atoms replicated. Atom features are packed as 4 groups
of 32 partitions (K=28 padded to 32) so the Z DMA uses all 128
partitions; matmul operands may only sit at partition offsets
{0, 32, 64}, so the 768-col 4th group shares offset 0 at zg columns
1536+. The W slab is replicated into the first 3 partition groups so
every (block, group) matmul has stationary and moving operands at the
same partition offset.
"""

import numpy as np
import ml_dtypes

import concourse.bacc as bacc
import concourse.tile as tile
from concourse import mybir
from concourse.bass_utils import run_bass_kernel_spmd

F32 = mybir.dt.float32
BF16 = mybir.dt.bfloat16
AX = mybir.AxisListType
OP = mybir.AluOpType
ACTF = mybir.ActivationFunctionType

NCORES = 8
NF = 3072            # frames total
NFS = NF // NCORES   # 384 frames per core
NB = NFS // 128      # 3 stationary blocks per core
NA = 5376            # atoms (replicated on every core)
K = 28               # feature dim
# atom groups: (partition offset, zg column offset, columns). Matmul
# operands may only sit at partition offsets {0, 32, 64}, so the 4th
# 768-col group shares partition offset 0 at zg columns 1536+.
GROUPS = [(0, 0, 1536), (32, 0, 1536), (64, 0, 1536), (0, 1536, 768)]
GC = 2304            # zg tile columns (1536 + 768 on partition group 0)
EPS = 1e-4
CLAMP2 = 100.0       # clamp on d^2 (= 10.0 on d)
ZSCALE = 10.0
NACC = 8             # ACT accumulator columns (2 + 2 + 4 spans)


def build_nc():
    nc = bacc.Bacc(None)

    # single input slab: cols [0, NFS) = W replicated (wr), cols
    # [NFS, NFS+GC) = atom feature groups (zg)
    wz_d = nc.declare_dram_parameter("wz", [128, NFS + GC], BF16,
                                     isOutput=False)
    out_d = nc.declare_dram_parameter("out", [128, NACC], F32, isOutput=True)

    with tile.TileContext(nc) as tc:
        with (
            tc.tile_pool(name="consts", bufs=1) as consts,
            tc.tile_pool(name="psum", bufs=2, space="PSUM") as psum_pool,
            tc.tile_pool(name="dpool", bufs=3) as dpool,
            tc.tile_pool(name="acts", bufs=2) as acts,
        ):
            wz = consts.tile([128, NFS + GC], BF16)
            wr = wz[:, 0:NFS]
            zg = wz[:, NFS:NFS + GC]
            # DMAs stay off the Scalar queue so ACT table loads overlap
            # them. DMA-A carries wr + the first zg piece (everything the
            # first matmuls need) so no small transfer gates on the slow
            # gpsimd queue; the group-3 tail columns (needed last) go there.
            nc.sync.dma_start(out=wz[:, 0:NFS + 1024],
                              in_=wz_d[:, 0:NFS + 1024])
            nc.sync.dma_start(out=wz[:, NFS + 1024:NFS + 1536],
                              in_=wz_d[:, NFS + 1024:NFS + 1536])
            nc.sync.dma_start(out=wz[:, NFS + 1536:NFS + GC],
                              in_=wz_d[:, NFS + 1536:NFS + GC])

            colacc = consts.tile([128, NACC], F32)
            bias_v = consts.tile([128, 1], F32)
            nc.vector.memset(bias_v[:], EPS)

            # Per-block tiles: (db col, cols, short?, pieces). The 768-col
            # group-3 work splits into a 512-col tile on a dedicated
            # 1-bank "short" tag plus a 256-col tile on the main ring,
            # giving the 1536-col tiles 3 effective PSUM slots in 7 banks
            # (all 8 banks allocated throttles every engine ~20%) --
            # this breaks the 2-buffer MM->DVE round-trip stall.
            TILES = [
                (0, 1536, False, [(0, 0, 512), (0, 512, 512), (0, 1024, 512)]),
                (1536, 1536, False, [(32, 0, 512), (32, 512, 512),
                                     (32, 1024, 512)]),
                (3072, 1536, False, [(64, 0, 512), (64, 512, 512),
                                     (64, 1024, 512)]),
                (4608, 512, True, [(0, 1536, 512)]),
                (5120, 256, False, [(0, 2048, 256)]),
            ]
            # ACT spans: half-blocks, except the last block runs finer
            # spans so the pipeline tail is short. Span (ti, start, width)
            # is emitted after the DVE clamp of tile ti. NOTE: schedule
            # reorderings and extra PSUM slots that increase engine
            # concurrency trip a ~20% chip-wide clock drop (power
            # governor) -- this ordering sits just under the threshold.
            HALF = [(1, 0, 2688), (4, 2688, 2688)]
            FINE = [(0, 0, 1536), (1, 1536, 1536), (2, 3072, 1536),
                    (4, 4608, 768)]
            idx = 0
            for b in range(NB):
                db = dpool.tile([128, NA], BF16, tag="d")
                spans = FINE if b == NB - 1 else HALF
                for ti, (tstart, cols, short, pieces) in enumerate(TILES):
                    lhsT = wr[:, 128 * b:128 * (b + 1)]
                    if short:
                        ps = psum_pool.tile([128, 512], F32, tag="short",
                                            bufs=1)
                    else:
                        ps = psum_pool.tile([128, 1536], F32, tag="main")
                    dst = 0
                    for (poff, zcol, w) in pieces:
                        nc.tensor.matmul(
                            ps[:, dst:dst + w],
                            wr[poff:poff + K, 128 * b:128 * (b + 1)],
                            zg[poff:poff + K, zcol:zcol + w],
                        )
                        dst += w
                    # clamp d2 to [0, 100] -> bf16 (NaN-proofs the sqrt)
                    nc.vector.tensor_scalar(
                        out=db[:, tstart:tstart + cols],
                        in0=ps[:, 0:cols],
                        scalar1=0.0,
                        scalar2=CLAMP2,
                        op0=OP.max,
                        op1=OP.min,
                    )
                    for (sg, start, width) in spans:
                        if sg != ti:
                            continue
                        s = acts.tile([128, 2688], BF16, tag="act")
                        nc.scalar.activation(
                            out=s[:, 0:width],
                            in_=db[:, start:start + width],
                            func=ACTF.Sqrt,
                            bias=bias_v[:, 0:1],
                            scale=1.0,
                            accum_out=colacc[:, idx:idx + 1],
                        )
                        idx += 1

            # ---- epilogue: DMA the raw accumulator columns; the host's
            # unshard step sums the 128 x NACC partials per core.
            # single_packet shortcuts descriptor generation for this tiny
            # transfer -- the exit barrier waits on its completion.
            # (Issuing from the Scalar queue instead measured +6us.) ----
            nc.sync.dma_start(out=out_d[:], in_=colacc[:], single_packet=True)

    nc.compile()
    return nc


_SYM = [(0, 0), (0, 1), (0, 2), (1, 1), (1, 2), (2, 2)]


def _features(inputs):
    """Host-side K=28 feature slabs W [28, NF] and Z [28, NA] (float64)."""
    f8 = np.float64
    pR = np.asarray(inputs["predicted_frames_R"], f8).reshape(NF, 3, 3)
    tR = np.asarray(inputs["true_frames_R"], f8).reshape(NF, 3, 3)
    pt = np.asarray(inputs["predicted_frames_t"], f8).reshape(NF, 3)
    tt = np.asarray(inputs["true_frames_t"], f8).reshape(NF, 3)
    pp = np.asarray(inputs["predicted_atom_positions"], f8).reshape(NA, 3)
    tp = np.asarray(inputs["true_atom_positions"], f8).reshape(NA, 3)

    w = np.einsum("fki,fk->fi", pR, pt) - np.einsum("fki,fk->fi", tR, tt)
    A = np.concatenate(
        [pR.transpose(0, 2, 1), -tR.transpose(0, 2, 1), -w[:, :, None]], axis=2
    )  # (NF, 3, 7)
    G = np.einsum("fki,fkj->fij", A, A)  # (NF, 7, 7)

    W = np.empty((K, NF), f8)
    Z = np.empty((K, NA), f8)
    r = 0
    for (i, j) in _SYM:
        W[r] = G[:, i, j] * (1.0 if i == j else 2.0)
        Z[r] = pp[:, i] * pp[:, j]
        r += 1
    for (i, j) in _SYM:
        W[r] = G[:, 3 + i, 3 + j] * (1.0 if i == j else 2.0)
        Z[r] = tp[:, i] * tp[:, j]
        r += 1
    for i in range(3):
        for j in range(3):
            W[r] = 2.0 * G[:, i, 3 + j]
            Z[r] = pp[:, i] * tp[:, j]
            r += 1
    for i in range(3):
        W[r] = 2.0 * G[:, i, 6]
        Z[r] = pp[:, i]
        r += 1
    for i in range(3):
        W[r] = 2.0 * G[:, 3 + i, 6]
        Z[r] = tp[:, i]
        r += 1
    W[27] = G[:, 6, 6]
    Z[27] = 1.0
    return W, Z


def prep_in_maps(inputs):
    """Full numpy inputs -> per-core input dicts + host-side norm info."""
    bf = ml_dtypes.bfloat16
    W, Z = _features(inputs)

    am = np.asarray(inputs["atom_mask"], np.float64).reshape(NA)
    mask_zero = am <= 0.5
    n_zero = int(mask_zero.sum())
    if n_zero:
        Z[:, mask_zero] = 0.0

    # zg: [128, GC]; group i covers atom cols [1536i, 1536i+cols) and
    # lives at (partition offset poff, zg column offset coff)
    zg = np.zeros((128, GC), bf)
    for i, (poff, coff, cols) in enumerate(GROUPS):
        zg[poff:poff + K, coff:coff + cols] = (
            Z[:, 1536 * i:1536 * i + cols].astype(bf)
        )

    in_maps = []
    for c in range(NCORES):
        Wc = W[:, c * NFS:(c + 1) * NFS].astype(bf)  # [28, 384]
        wzc = np.zeros((128, NFS + GC), bf)
        for poff in (0, 32, 64):
            wzc[poff:poff + K, 0:NFS] = Wc
        wzc[:, NFS:NFS + GC] = zg
        in_maps.append({"wz": wzc})

    norm = {
        "mask_sum": float(am.sum()),
        "pad_corr": 0.01 * float(NF) * n_zero,  # sqrt(eps) per zeroed pair
    }
    return in_maps, norm


_NC_CACHE = None


def _get_nc():
    global _NC_CACHE
    if _NC_CACHE is None:
        _NC_CACHE = build_nc()
    return _NC_CACHE


def kernel(**inputs):
    nc = _get_nc()
    in_maps, norm = prep_in_maps(inputs)
    r = run_bass_kernel_spmd(nc, in_maps, core_ids=list(range(NCORES)))
    total = 0.0
    for i in range(NCORES):
        total += float(np.asarray(r.results[i]["out"], np.float64).sum())
    total -= norm["pad_corr"]
    res = total / (ZSCALE * (float(NF) + EPS) * (EPS + norm["mask_sum"]))
    return np.array([res], dtype=np.float32)


# revision 62
# speedup vs baseline: 1.1796x; 1.1796x over previous
"""AllAtomFAPE loss kernel for Trainium2 (8 NeuronCores, SPMD).

Problem: b=1, N=384 res, F=8 frames/res -> NF=3072 frames; A=14 atoms/res
-> NA=5376 atoms. Output: scalar (shape (1,)) masked clamped FAPE.

Algorithm (K=28 Gram factorization, host-precomputed features):
  lp - lt = A_f x_a with A_f = [pR^T | -tR^T | -w_f] (3x7),
  w_f = pR^T pt - tR^T tt, x_a = [pp; tp; 1] (7,). So
    d2(f,a) = x_a^T G_f x_a,  G_f = A_f^T A_f (7x7 PSD)
  which is a K=28 dot product between frame features W[:,f] (the 28
  unique entries of G, off-diagonals doubled) and atom features Z[:,a]
  (the matching monomials of x_a). Both feature slabs are computed on
  the host in float64 and rounded to bf16; the device does only the
  O(NF*NA) pairwise work, pipelined per (block, group) tile:
    PE   : d2 tile = W_blk^T @ Z  (bf16 matmul, f32 PSUM, 512-col moving)
    DVE  : clamp(d2, 0, 100) -> bf16 block buffer (clamp commutes with
           sqrt: min(sqrt(x+eps),10) = sqrt(min(x,100)+eps); the
           max(.,0) guards bf16-rounding-induced negative d2 from
           NaN-ing the sqrt). This pass is the ~19us critical stream;
           it must run on DVE (GpSimd cannot read PSUM, ACT cannot
           clamp) at 1 elem/cycle (f32 input forbids the 2x modes).
    ACT  : sqrt(x + eps) + per-partition accumulate, over half-block
           spans (per-tile spans in the last block shorten the tail)
  Epilogue: DMA the raw [128, 8] accumulator columns per core straight
  from SBUF (no on-device reduce -- shortens the serial tail); the
  host's unshard step sums the 8 per-core slabs and normalizes (binary
  atom masks fold in by zeroing Z columns + a sqrt(eps) correction).
  Steady state is PE-throughput-bound (~600ns per 512-col bf16 matmul
  + ldweights reload per call); fp8 DoubleRow would halve that but
  costs a systematic fp8-rounding bias in d^2.

Sharding: frames sharded across the 8 cores (384 each = 3 stationary
blocks of 128); atoms replicated. Atom features are packed as 4 groups
of 32 partitions (K=28 padded to 32) so the Z DMA uses all 128
partitions; matmul operands may only sit at partition offsets
{0, 32, 64}, so the 768-col 4th group shares offset 0 at zg columns
1536+. The W slab is replicated into the first 3 partition groups so
every (block, group) matmul has stationary and moving operands at the
same partition offset.
"""

import numpy as np
import ml_dtypes

import concourse.bacc as bacc
import concourse.tile as tile
from concourse import mybir
from concourse.bass_utils import run_bass_kernel_spmd

F32 = mybir.dt.float32
BF16 = mybir.dt.bfloat16
AX = mybir.AxisListType
OP = mybir.AluOpType
ACTF = mybir.ActivationFunctionType

NCORES = 8
NF = 3072            # frames total
NFS = NF // NCORES   # 384 frames per core
NB = NFS // 128      # 3 stationary blocks per core
NA = 5376            # atoms (replicated on every core)
K = 28               # feature dim
# atom groups: (partition offset, zg column offset, columns). Matmul
# operands may only sit at partition offsets {0, 32, 64}, so the 4th
# 768-col group shares partition offset 0 at zg columns 1536+.
GROUPS = [(0, 0, 1536), (32, 0, 1536), (64, 0, 1536), (0, 1536, 768)]
GC = 2304            # zg tile columns (1536 + 768 on partition group 0)
EPS = 1e-4
CLAMP2 = 100.0       # clamp on d^2 (= 10.0 on d)
ZSCALE = 10.0
NACC = 8             # ACT accumulator columns (2 + 2 + 4 spans)


def build_nc():
    nc = bacc.Bacc(None)

    # single input slab: cols [0, NFS) = W replicated (wr), cols
    # [NFS, NFS+GC) = atom feature groups (zg)
    wz_d = nc.declare_dram_parameter("wz", [128, NFS + GC], BF16,
                                     isOutput=False)
    out_d = nc.declare_dram_parameter("out", [128, NACC], F32, isOutput=True)

    with tile.TileContext(nc) as tc:
        with (
            tc.tile_pool(name="consts", bufs=1) as consts,
            tc.tile_pool(name="psum", bufs=2, space="PSUM") as psum_pool,
            tc.tile_pool(name="dpool", bufs=3) as dpool,
            tc.tile_pool(name="acts", bufs=2) as acts,
        ):
            wz = consts.tile([128, NFS + GC], BF16)
            wr = wz[:, 0:NFS]
            zg = wz[:, NFS:NFS + GC]
            # DMAs stay off the Scalar queue so ACT table loads overlap
            # them. DMA-A carries wr + the first zg piece (everything the
            # first matmuls need) so no small transfer gates on the slow
            # gpsimd queue; the group-3 tail columns (needed last) go there.
            nc.sync.dma_start(out=wz[:, 0:NFS + 1024],
                              in_=wz_d[:, 0:NFS + 1024])
            nc.sync.dma_start(out=wz[:, NFS + 1024:NFS + 1536],
                              in_=wz_d[:, NFS + 1024:NFS + 1536])
            nc.sync.dma_start(out=wz[:, NFS + 1536:NFS + GC],
                              in_=wz_d[:, NFS + 1536:NFS + GC])

            colacc = consts.tile([128, NACC], F32)
            bias_v = consts.tile([128, 1], F32)
            nc.vector.memset(bias_v[:], EPS)

            # Per-block tiles: (db col, cols, short?, pieces). The 768-col
            # group-3 work splits into a 512-col tile on a dedicated
            # 1-bank "short" tag plus a 256-col tile on the main ring,
            # giving the 1536-col tiles 3 effective PSUM slots in 7 banks
            # (all 8 banks allocated throttles every engine ~20%) --
            # this breaks the 2-buffer MM->DVE round-trip stall.
            TILES = [
                (0, 1536, False, [(0, 0, 512), (0, 512, 512), (0, 1024, 512)]),
                (1536, 1536, False, [(32, 0, 512), (32, 512, 512),
                                     (32, 1024, 512)]),
                (3072, 1536, False, [(64, 0, 512), (64, 512, 512),
                                     (64, 1024, 512)]),
                (4608, 768, True, [(0, 1536, 512), (0, 2048, 256)]),
            ]
            # ACT spans: half-blocks, except the last block runs finer
            # spans so the pipeline tail is short. Span (ti, start, width)
            # is emitted after the DVE clamp of tile ti. NOTE: schedule
            # reorderings and extra PSUM slots that increase engine
            # concurrency trip a ~20% chip-wide clock drop (power
            # governor) -- this ordering sits just under the threshold.
            HALF = [(1, 0, 2688), (3, 2688, 2688)]
            FINE = [(0, 0, 1536), (1, 1536, 1536), (2, 3072, 1536),
                    (3, 4608, 768)]
            idx = 0
            for b in range(NB):
                db = dpool.tile([128, NA], BF16, tag="d")
                spans = FINE if b == NB - 1 else HALF
                for ti, (tstart, cols, short, pieces) in enumerate(TILES):
                    lhsT = wr[:, 128 * b:128 * (b + 1)]
                    if short:
                        ps = psum_pool.tile([128, 768], F32, tag="short",
                                            bufs=1)
                    else:
                        ps = psum_pool.tile([128, 1536], F32, tag="main")
                    dst = 0
                    for (poff, zcol, w) in pieces:
                        nc.tensor.matmul(
                            ps[:, dst:dst + w],
                            wr[poff:poff + K, 128 * b:128 * (b + 1)],
                            zg[poff:poff + K, zcol:zcol + w],
                        )
                        dst += w
                    # clamp d2 to [0, 100] -> bf16 (NaN-proofs the sqrt)
                    nc.vector.tensor_scalar(
                        out=db[:, tstart:tstart + cols],
                        in0=ps[:, 0:cols],
                        scalar1=0.0,
                        scalar2=CLAMP2,
                        op0=OP.max,
                        op1=OP.min,
                    )
                    for (sg, start, width) in spans:
                        if sg != ti:
                            continue
                        s = acts.tile([128, 2688], BF16, tag="act")
                        nc.scalar.activation(
                            out=s[:, 0:width],
                            in_=db[:, start:start + width],
                            func=ACTF.Sqrt,
                            bias=bias_v[:, 0:1],
                            scale=1.0,
                            accum_out=colacc[:, idx:idx + 1],
                        )
                        idx += 1

            # ---- epilogue: DMA the raw accumulator columns; the host's
            # unshard step sums the 128 x NACC partials per core.
            # single_packet shortcuts descriptor generation for this tiny
            # transfer -- the exit barrier waits on its completion.
            # (Issuing from the Scalar queue instead measured +6us.) ----
            nc.sync.dma_start(out=out_d[:], in_=colacc[:], single_packet=True)

    nc.compile()
    return nc


_SYM = [(0, 0), (0, 1), (0, 2), (1, 1), (1, 2), (2, 2)]


def _features(inputs):
    """Host-side K=28 feature slabs W [28, NF] and Z [28, NA] (float64)."""
    f8 = np.float64
    pR = np.asarray(inputs["predicted_frames_R"], f8).reshape(NF, 3, 3)
    tR = np.asarray(inputs["true_frames_R"], f8).reshape(NF, 3, 3)
    pt = np.asarray(inputs["predicted_frames_t"], f8).reshape(NF, 3)
    tt = np.asarray(inputs["true_frames_t"], f8).reshape(NF, 3)
    pp = np.asarray(inputs["predicted_atom_positions"], f8).reshape(NA, 3)
    tp = np.asarray(inputs["true_atom_positions"], f8).reshape(NA, 3)

    w = np.einsum("fki,fk->fi", pR, pt) - np.einsum("fki,fk->fi", tR, tt)
    A = np.concatenate(
        [pR.transpose(0, 2, 1), -tR.transpose(0, 2, 1), -w[:, :, None]], axis=2
    )  # (NF, 3, 7)
    G = np.einsum("fki,fkj->fij", A, A)  # (NF, 7, 7)

    W = np.empty((K, NF), f8)
    Z = np.empty((K, NA), f8)
    r = 0
    for (i, j) in _SYM:
        W[r] = G[:, i, j] * (1.0 if i == j else 2.0)
        Z[r] = pp[:, i] * pp[:, j]
        r += 1
    for (i, j) in _SYM:
        W[r] = G[:, 3 + i, 3 + j] * (1.0 if i == j else 2.0)
        Z[r] = tp[:, i] * tp[:, j]
        r += 1
    for i in range(3):
        for j in range(3):
            W[r] = 2.0 * G[:, i, 3 + j]
            Z[r] = pp[:, i] * tp[:, j]
            r += 1
    for i in range(3):
        W[r] = 2.0 * G[:, i, 6]
        Z[r] = pp[:, i]
        r += 1
    for i in range(3):
        W[r] = 2.0 * G[:, 3 + i, 6]
        Z[r] = tp[:, i]
        r += 1
    W[27] = G[:, 6, 6]
    Z[27] = 1.0
    return W, Z


def prep_in_maps(inputs):
    """Full numpy inputs -> per-core input dicts + host-side norm info."""
    bf = ml_dtypes.bfloat16
    W, Z = _features(inputs)

    am = np.asarray(inputs["atom_mask"], np.float64).reshape(NA)
    mask_zero = am <= 0.5
    n_zero = int(mask_zero.sum())
    if n_zero:
        Z[:, mask_zero] = 0.0

    # zg: [128, GC]; group i covers atom cols [1536i, 1536i+cols) and
    # lives at (partition offset poff, zg column offset coff)
    zg = np.zeros((128, GC), bf)
    for i, (poff, coff, cols) in enumerate(GROUPS):
        zg[poff:poff + K, coff:coff + cols] = (
            Z[:, 1536 * i:1536 * i + cols].astype(bf)
        )

    in_maps = []
    for c in range(NCORES):
        Wc = W[:, c * NFS:(c + 1) * NFS].astype(bf)  # [28, 384]
        wzc = np.zeros((128, NFS + GC), bf)
        for poff in (0, 32, 64):
            wzc[poff:poff + K, 0:NFS] = Wc
        wzc[:, NFS:NFS + GC] = zg
        in_maps.append({"wz": wzc})

    norm = {
        "mask_sum": float(am.sum()),
        "pad_corr": 0.01 * float(NF) * n_zero,  # sqrt(eps) per zeroed pair
    }
    return in_maps, norm


_NC_CACHE = None


def _get_nc():
    global _NC_CACHE
    if _NC_CACHE is None:
        _NC_CACHE = build_nc()
    return _NC_CACHE


def kernel(**inputs):
    nc = _get_nc()
    in_maps, norm = prep_in_maps(inputs)
    r = run_bass_kernel_spmd(nc, in_maps, core_ids=list(range(NCORES)))
    total = 0.0
    for i in range(NCORES):
        total += float(np.asarray(r.results[i]["out"], np.float64).sum())
    total -= norm["pad_corr"]
    res = total / (ZSCALE * (float(NF) + EPS) * (EPS + norm["mask_sum"]))
    return np.array([res], dtype=np.float32)


# revision 65
# speedup vs baseline: 1.2126x; 1.0280x over previous
"""AllAtomFAPE loss kernel for Trainium2 (8 NeuronCores, SPMD).

Problem: b=1, N=384 res, F=8 frames/res -> NF=3072 frames; A=14 atoms/res
-> NA=5376 atoms. Output: scalar (shape (1,)) masked clamped FAPE.

Algorithm (K=28 Gram factorization, host-precomputed features):
  lp - lt = A_f x_a with A_f = [pR^T | -tR^T | -w_f] (3x7),
  w_f = pR^T pt - tR^T tt, x_a = [pp; tp; 1] (7,). So
    d2(f,a) = x_a^T G_f x_a,  G_f = A_f^T A_f (7x7 PSD)
  which is a K=28 dot product between frame features W[:,f] (the 28
  unique entries of G, off-diagonals doubled) and atom features Z[:,a]
  (the matching monomials of x_a). Both feature slabs are computed on
  the host in float64 and rounded to bf16; the device does only the
  O(NF*NA) pairwise work, pipelined per (block, group) tile:
    PE   : d2 tile = W_blk^T @ Z  (bf16 matmul, f32 PSUM, 512-col moving)
    DVE  : clamp(d2, 0, 100) -> bf16 block buffer (clamp commutes with
           sqrt: min(sqrt(x+eps),10) = sqrt(min(x,100)+eps); the
           max(.,0) guards bf16-rounding-induced negative d2 from
           NaN-ing the sqrt). This pass is the ~19us critical stream;
           it must run on DVE (GpSimd cannot read PSUM, ACT cannot
           clamp) at 1 elem/cycle (f32 input forbids the 2x modes).
    ACT  : sqrt(x + eps) + per-partition accumulate, over half-block
           spans (per-tile spans in the last block shorten the tail)
  Epilogue: DMA the raw [128, 8] accumulator columns per core straight
  from SBUF (no on-device reduce -- shortens the serial tail); the
  host's unshard step sums the 8 per-core slabs and normalizes (binary
  atom masks fold in by zeroing Z columns + a sqrt(eps) correction).
  Steady state is PE-throughput-bound (~600ns per 512-col bf16 matmul
  + ldweights reload per call); fp8 DoubleRow would halve that but
  costs a systematic fp8-rounding bias in d^2.

Sharding: frames sharded across the 8 cores (384 each = 3 stationary
blocks of 128); atoms replicated. Atom features are packed as 4 groups
of 32 partitions (K=28 padded to 32) so the Z DMA uses all 128
partitions; matmul operands may only sit at partition offsets
{0, 32, 64}, so the 768-col 4th group shares offset 0 at zg columns
1536+. The W slab is replicated into the first 3 partition groups so
every (block, group) matmul has stationary and moving operands at the
same partition offset.
"""

import numpy as np
import ml_dtypes

import concourse.bacc as bacc
import concourse.tile as tile
from concourse import mybir
from concourse.bass_utils import run_bass_kernel_spmd

F32 = mybir.dt.float32
BF16 = mybir.dt.bfloat16
AX = mybir.AxisListType
OP = mybir.AluOpType
ACTF = mybir.ActivationFunctionType

NCORES = 8
NF = 3072            # frames total
NFS = NF // NCORES   # 384 frames per core
NB = NFS // 128      # 3 stationary blocks per core
NA = 5376            # atoms (replicated on every core)
K = 28               # feature dim
# atom groups: (partition offset, zg column offset, columns). Matmul
# operands may only sit at partition offsets {0, 32, 64}, so the 4th
# 768-col group shares partition offset 0 at zg columns 1536+.
GROUPS = [(0, 0, 1536), (32, 0, 1536), (64, 0, 1536), (0, 1536, 768)]
GC = 2304            # zg tile columns (1536 + 768 on partition group 0)
EPS = 1e-4
CLAMP2 = 100.0       # clamp on d^2 (= 10.0 on d)
ZSCALE = 10.0
NACC = 8             # ACT accumulator columns (2 + 2 + 4 spans)


def build_nc():
    nc = bacc.Bacc(None)

    # single input slab: cols [0, NFS) = W replicated (wr), cols
    # [NFS, NFS+GC) = atom feature groups (zg)
    wz_d = nc.declare_dram_parameter("wz", [128, NFS + GC], BF16,
                                     isOutput=False)
    out_d = nc.declare_dram_parameter("out", [128, NACC], F32, isOutput=True)

    with tile.TileContext(nc) as tc:
        with (
            tc.tile_pool(name="consts", bufs=1) as consts,
            tc.tile_pool(name="psum", bufs=2, space="PSUM") as psum_pool,
            tc.tile_pool(name="dpool", bufs=3) as dpool,
            tc.tile_pool(name="acts", bufs=2) as acts,
        ):
            wz = consts.tile([128, NFS + GC], BF16)
            wr = wz[:, 0:NFS]
            zg = wz[:, NFS:NFS + GC]
            # DMAs stay off the Scalar queue so ACT table loads overlap
            # them. DMA-A carries wr + the first zg piece (everything the
            # first matmuls need) so no small transfer gates on the slow
            # gpsimd queue; the group-3 tail columns (needed last) go there.
            nc.sync.dma_start(out=wz[:, 0:NFS + 1024],
                              in_=wz_d[:, 0:NFS + 1024])
            nc.sync.dma_start(out=wz[:, NFS + 1024:NFS + 1536],
                              in_=wz_d[:, NFS + 1024:NFS + 1536])
            nc.sync.dma_start(out=wz[:, NFS + 1536:NFS + GC],
                              in_=wz_d[:, NFS + 1536:NFS + GC])

            colacc = consts.tile([128, NACC], F32)
            bias_v = consts.tile([128, 1], F32)
            nc.vector.memset(bias_v[:], EPS)

            # Per-block tiles: (db col, cols, short?, pieces). The 768-col
            # group-3 work splits into a 512-col tile on a dedicated
            # 1-bank "short" tag plus a 256-col tile on the main ring,
            # giving the 1536-col tiles 3 effective PSUM slots in 7 banks
            # (all 8 banks allocated throttles every engine ~20%) --
            # this breaks the 2-buffer MM->DVE round-trip stall.
            TILES = [
                (0, 1536, False, [(0, 0, 512), (0, 512, 512), (0, 1024, 512)]),
                (1536, 1536, False, [(32, 0, 512), (32, 512, 512),
                                     (32, 1024, 512)]),
                (3072, 1536, False, [(64, 0, 512), (64, 512, 512),
                                     (64, 1024, 512)]),
                (4608, 512, True, [(0, 1536, 512)]),
                (5120, 256, False, [(0, 2048, 256)]),
            ]
            # ACT spans: half-blocks, except the last block runs finer
            # spans so the pipeline tail is short. Span (ti, start, width)
            # is emitted after the DVE clamp of tile ti. NOTE: schedule
            # reorderings and extra PSUM slots that increase engine
            # concurrency trip a ~20% chip-wide clock drop (power
            # governor) -- this ordering sits just under the threshold.
            HALF = [(1, 0, 2688), (4, 2688, 2688)]
            FINE = [(0, 0, 1536), (1, 1536, 1536), (2, 3072, 1536),
                    (4, 4608, 768)]
            idx = 0
            for b in range(NB):
                db = dpool.tile([128, NA], BF16, tag="d")
                spans = FINE if b == NB - 1 else HALF
                for ti, (tstart, cols, short, pieces) in enumerate(TILES):
                    lhsT = wr[:, 128 * b:128 * (b + 1)]
                    if short:
                        ps = psum_pool.tile([128, 512], F32, tag="short",
                                            bufs=1)
                    else:
                        ps = psum_pool.tile([128, 1536], F32, tag="main")
                    dst = 0
                    for (poff, zcol, w) in pieces:
                        nc.tensor.matmul(
                            ps[:, dst:dst + w],
                            wr[poff:poff + K, 128 * b:128 * (b + 1)],
                            zg[poff:poff + K, zcol:zcol + w],
                        )
                        dst += w
                    # clamp d2 to [0, 100] -> bf16 (NaN-proofs the sqrt)
                    nc.vector.tensor_scalar(
                        out=db[:, tstart:tstart + cols],
                        in0=ps[:, 0:cols],
                        scalar1=0.0,
                        scalar2=CLAMP2,
                        op0=OP.max,
                        op1=OP.min,
                    )
                    for (sg, start, width) in spans:
                        if sg != ti:
                            continue
                        s = acts.tile([128, 2688], BF16, tag="act")
                        nc.scalar.activation(
                            out=s[:, 0:width],
                            in_=db[:, start:start + width],
                            func=ACTF.Sqrt,
                            bias=bias_v[:, 0:1],
                            scale=1.0,
                            accum_out=colacc[:, idx:idx + 1],
                        )
                        idx += 1

            # ---- epilogue: DMA the raw accumulator columns; the host's
            # unshard step sums the 128 x NACC partials per core.
            # single_packet shortcuts descriptor generation for this tiny
            # transfer -- the exit barrier waits on its completion.
            # (Issuing from the Scalar queue instead measured +6us.) ----
            nc.sync.dma_start(out=out_d[:], in_=colacc[:], single_packet=True)

    nc.compile()
    return nc


_SYM = [(0, 0), (0, 1), (0, 2), (1, 1), (1, 2), (2, 2)]


def _features(inputs):
    """Host-side K=28 feature slabs W [28, NF] and Z [28, NA] (float64)."""
    f8 = np.float64
    pR = np.asarray(inputs["predicted_frames_R"], f8).reshape(NF, 3, 3)
    tR = np.asarray(inputs["true_frames_R"], f8).reshape(NF, 3, 3)
    pt = np.asarray(inputs["predicted_frames_t"], f8).reshape(NF, 3)
    tt = np.asarray(inputs["true_frames_t"], f8).reshape(NF, 3)
    pp = np.asarray(inputs["predicted_atom_positions"], f8).reshape(NA, 3)
    tp = np.asarray(inputs["true_atom_positions"], f8).reshape(NA, 3)

    w = np.einsum("fki,fk->fi", pR, pt) - np.einsum("fki,fk->fi", tR, tt)
    A = np.concatenate(
        [pR.transpose(0, 2, 1), -tR.transpose(0, 2, 1), -w[:, :, None]], axis=2
    )  # (NF, 3, 7)
    G = np.einsum("fki,fkj->fij", A, A)  # (NF, 7, 7)

    W = np.empty((K, NF), f8)
    Z = np.empty((K, NA), f8)
    r = 0
    for (i, j) in _SYM:
        W[r] = G[:, i, j] * (1.0 if i == j else 2.0)
        Z[r] = pp[:, i] * pp[:, j]
        r += 1
    for (i, j) in _SYM:
        W[r] = G[:, 3 + i, 3 + j] * (1.0 if i == j else 2.0)
        Z[r] = tp[:, i] * tp[:, j]
        r += 1
    for i in range(3):
        for j in range(3):
            W[r] = 2.0 * G[:, i, 3 + j]
            Z[r] = pp[:, i] * tp[:, j]
            r += 1
    for i in range(3):
        W[r] = 2.0 * G[:, i, 6]
        Z[r] = pp[:, i]
        r += 1
    for i in range(3):
        W[r] = 2.0 * G[:, 3 + i, 6]
        Z[r] = tp[:, i]
        r += 1
    W[27] = G[:, 6, 6]
    Z[27] = 1.0
    return W, Z


def prep_in_maps(inputs):
    """Full numpy inputs -> per-core input dicts + host-side norm info."""
    bf = ml_dtypes.bfloat16
    W, Z = _features(inputs)

    am = np.asarray(inputs["atom_mask"], np.float64).reshape(NA)
    mask_zero = am <= 0.5
    n_zero = int(mask_zero.sum())
    if n_zero:
        Z[:, mask_zero] = 0.0

    # zg: [128, GC]; group i covers atom cols [1536i, 1536i+cols) and
    # lives at (partition offset poff, zg column offset coff)
    zg = np.zeros((128, GC), bf)
    for i, (poff, coff, cols) in enumerate(GROUPS):
        zg[poff:poff + K, coff:coff + cols] = (
            Z[:, 1536 * i:1536 * i + cols].astype(bf)
        )

    in_maps = []
    for c in range(NCORES):
        Wc = W[:, c * NFS:(c + 1) * NFS].astype(bf)  # [28, 384]
        wzc = np.zeros((128, NFS + GC), bf)
        for poff in (0, 32, 64):
            wzc[poff:poff + K, 0:NFS] = Wc
        wzc[:, NFS:NFS + GC] = zg
        in_maps.append({"wz": wzc})

    norm = {
        "mask_sum": float(am.sum()),
        "pad_corr": 0.01 * float(NF) * n_zero,  # sqrt(eps) per zeroed pair
    }
    return in_maps, norm


_NC_CACHE = None


def _get_nc():
    global _NC_CACHE
    if _NC_CACHE is None:
        _NC_CACHE = build_nc()
    return _NC_CACHE


def kernel(**inputs):
    nc = _get_nc()
    in_maps, norm = prep_in_maps(inputs)
    r = run_bass_kernel_spmd(nc, in_maps, core_ids=list(range(NCORES)))
    total = 0.0
    for i in range(NCORES):
        total += float(np.asarray(r.results[i]["out"], np.float64).sum())
    total -= norm["pad_corr"]
    res = total / (ZSCALE * (float(NF) + EPS) * (EPS + norm["mask_sum"]))
    return np.array([res], dtype=np.float32)
